# revision 90
# baseline (speedup 1.0000x reference)
"""Linear-chain CRF partition function (log Z) on 8 Trainium2 NeuronCores, v4.

Exp-domain recurrence p' = (ET^T p) * f as 128x128x512 PE matmuls plus
elementwise multiplies, spread over three engine paths per core:

  - V streams: DVE tensor_mul reads the f32 PSUM matmul output directly
    (658ns/slot), fp8 features from SBUF.
  - G streams: ACT copies PSUM->SBUF bf16 (570ns), GPSIMD multiplies
    all-SBUF (1111ns; it has no PSUM port).
  - X log-domain streams: ACT Exp -> PE matmul -> ACT Ln -> PE
    identity-matmuls add ln(q) + raw feats into a PSUM group; the feature
    "multiply" costs (idle) PE time instead of DVE/Pool.

The 1024-step scan splits into 18 chains/core x 8 cores; the first KHOST
steps are folded on the host in f64 (coverage parity + the exact-init
chain).  Ones-init chains converge to the true filtering direction at
~50x/step contraction, and their first two pre-feature states are RANK-1
(E^T 1 and E^T E^T 1 broadcast), so the host ships c2 = ET.T(ET.T 1) in
the consts blob: one ACT broadcast replaces every warmup matmul/copy and
every slot-1 matmul, and the slot-1 start-colsums are host constants.
Chains stitch via colsum ratios computed as transposed PE sums into one
PSUM tile; logs happen on the host.

Every engine op is placed by a static list-scheduler (_make_plan) that
mirrors the TimelineSim cost model (durations, completion latencies,
serial 625ns/transfer HWDGE, in-order engines with single wait slots);
per-engine emission order == schedule order, which removes head-of-line
blocking on the real single-wait-slot queues.  Features are fp8e4m3
exp(feat-3) (raw feats for the log path), packed host-side into ONE
need-ordered DRAM tensor so a handful of large DMAs feed all streams in
consumption order (DVE retires a block per 658ns, GPSIMD per 1111ns).
"""

import numpy as np
import ml_dtypes

import concourse.bacc as bacc
import concourse.bass as bass
import concourse.tile as tile
from concourse import mybir
from concourse._compat import with_exitstack
from concourse.bass_utils import run_bass_kernel_spmd

B, S, T2 = 256, 1024, 128
NCORES = 8
SHIFT = 3.0           # exp-domain chains use f = exp(feat - SHIFT)
BX = -16.0 / 3.0      # log-domain chains: per-slot exp bias
import os as _os
W = 1                 # warmup slots per chain (direction contracts ~50x/step)

# per-stream (kind, chains, length): "D2x18" = D kind, 2 chains (512-wide),
# 18 slots.  g=4 streams are PAIRED: 1024-wide tiles spanning two PSUM banks
# so ACT copies / Ln / Exp amortize their fixed PSUM-access cost; matmuls
# still write per-512 halves (PSUM bank limit).  Lengths tuned so every
# engine drains at the same time.
_ST = _os.environ.get("KV3_ST", "V2x12,V2x11,V2x11,G2x6,G2x6,G2x6,G2x6,X2x6,X2x5")
STREAMS = []
for s in _ST.split(","):
    kind, rest = s[0], s[1:]
    g, l = (int(x) for x in rest.split("x"))
    STREAMS.append((kind, g, l))
E_IDX = [i for i, (k, _, _) in enumerate(STREAMS) if k in "EVG"]
X_IDX = [i for i, (k, _, _) in enumerate(STREAMS) if k == "X"]
D_IDX = E_IDX  # first E stream carries the exact init

# per-slot multiply implementation for E/V/G streams: "V" = DVE tensor_mul
# straight from PSUM (658ns); "G" = ACT copy to SBUF bf16 (570ns) + GPSIMD
# tensor_mul (1111ns).  Kind V/G fixes the whole stream; kind E mixes by
# Bresenham at the engine-balance ratio with a per-stream phase so
# same-impl slots don't align across streams.
_FV = float(_os.environ.get("KV5_FV", "0.628"))
_IMPL = {}
for _i, (_k, _g, _l) in enumerate(STREAMS):
    if _k in ("V", "G"):
        for _s in range(_l):
            _IMPL[(_i, _s)] = _k
        continue
    if _k != "E":
        continue
    acc = (_i * 0.382) % 1.0
    for _s in range(_l):
        acc += _FV
        if acc >= 1.0:
            acc -= 1.0
            _IMPL[(_i, _s)] = "V"
        else:
            _IMPL[(_i, _s)] = "G"


def _is_fw(i):
    """Stream keeps features in its warmup slot (chain0's exact-init
    stream only).  All other streams' warmup applies just the transition
    matrix -- the warmup scale factor cancels in the colsum-ratio
    stitching, and direction convergence comes from E^T mixing, so the
    feature multiply there is pure overhead (and its features need not
    be DMA'd at all)."""
    return i == D_IDX[0]


def _flen(i):
    kind, g, l = STREAMS[i]
    return l if _is_fw(i) else l - W


def _fslot(i, s):
    return s if _is_fw(i) else s - W
COV = sum(g * (l - W) for _, g, l in STREAMS)
KHOST = S - W - NCORES * COV
assert 5 <= KHOST <= 64, KHOST
MAXG = max(g for _, g, _ in STREAMS)
CPC = sum(g for _, g, _ in STREAMS)      # chains per core
NCH = NCORES * CPC
BF16, F32, FP8 = mybir.dt.bfloat16, mybir.dt.float32, mybir.dt.float8e4
NPBF = ml_dtypes.bfloat16
NPF8 = ml_dtypes.float8_e4m3
AF = mybir.ActivationFunctionType

# consts blob layout (bf16, [T2, CT_W]): ET first so matmuls start early;
# the exact-init PINIT comes as a separate later tensor (only stream D0
# waits on it; every other stream starts from an on-chip memset of ones).
CT_ET = 0             # [128] exp(trans).T
CT_GE = 128           # [2]  col0=ones col1=exp(trans[END])
CT_ID = 130           # [128] identity
CT_C2 = 258           # [1]  c2 = ET.T @ (ET.T @ 1): slot-1 q for ones-init
CT_LX = 259           # [1]  ln(c2) + BX: slot-1 ln(q) for the X chain
CT_W = 260

BLK0 = np.cumsum([0] + [g for _, g, _ in STREAMS])[:-1]


def _plan():
    starts = []
    t = KHOST + W  # chain0's warmup slots cover steps KHOST..KHOST+W-1 exactly
    for _ in range(NCORES):
        for kind, g, l in STREAMS:
            for _ in range(g):
                starts.append(t)
                t += l - W
    assert t == S, t
    return starts


STARTS = _plan()


_CADENCE = {"E": 2300.0, "V": 2050.0, "G": 3400.0, "X": 2500.0}


def _forder():
    """All (stream, feature-slot) blocks in estimated need order, with the
    column offset of each block in the packed feature layout.  Need times
    come from per-ENGINE global consumption rank: the DVE retires one V
    block per 658ns, GPSIMD one G block per 1111ns, the X chain one per
    ~2500ns -- blocks of different streams of the same impl interleave
    round-robin."""
    per_impl = {"V": [], "G": [], "X": []}
    for i, (kind, g, l) in enumerate(STREAMS):
        for fs in range(_flen(i)):
            cs = fs + (0 if _is_fw(i) else W)
            im = kind if kind == "X" else _IMPL[(i, cs)]
            per_impl[im].append((fs, i))
    rate = {"V": 658.0, "G": 1111.0, "X": 2500.0}
    base = {"V": float(_os.environ.get("KV5_BV", "4000")),
            "G": float(_os.environ.get("KV5_BG", "4400")),
            "X": float(_os.environ.get("KV5_BX", "4700"))}
    items = []
    for im, blocks in per_impl.items():
        blocks.sort()
        for rank, (fs, i) in enumerate(blocks):
            items.append((base[im] + rank * rate[im], i, fs))
    items.sort()
    off, offs = 0, {}
    for need, i, fs in items:
        offs[(i, fs)] = off
        off += STREAMS[i][1] * B
    return items, offs, off


FORDER, FOFFS, FCOLS = _forder()


def _feature_chunks():
    """Split the need-ordered packed feature layout into a few large DMA
    ranges: the serial HWDGE costs 625ns per transfer, so all streams'
    early slots ride in one first chunk."""
    c0 = int(_os.environ.get("KV4_C0", "5"))    # blocks in first chunk
    c1 = int(_os.environ.get("KV4_C1", "5"))    # optional second chunk
    cn = int(_os.environ.get("KV4_CN", "10"))   # blocks per later chunk
    n = len(FORDER)
    cuts = [0, min(c0, n)]
    if c1:
        cuts.append(min(c0 + c1, n))
    while cuts[-1] < n:
        cuts.append(min(cuts[-1] + cn, n))
    chunks = []   # (col_a, col_b, need)
    for a, b in zip(cuts, cuts[1:]):
        col_a = FOFFS[FORDER[a][1], FORDER[a][2]]
        lastn, lasti, lastfs = FORDER[b - 1]
        col_b = FOFFS[(lasti, lastfs)] + STREAMS[lasti][1] * B
        chunks.append((col_a, col_b, FORDER[a][0]))
    return chunks


def _norm_oid(oid):
    return tuple(oid)


def _prov_sum_done():
    """Per-stream predicted sum1 completion from a schedule of the plan
    without stage/out ops (cached; deterministic)."""
    if "_PROV" in _PROV_CACHE:
        return _PROV_CACHE["_PROV"]
    import os as _o
    _o.environ["KV7_PROV"] = "1"
    try:
        _make_plan()
    finally:
        del _o.environ["KV7_PROV"]
    return _PROV_CACHE["_PROV"]


_PROV_CACHE = {}


# A converged emission order (from closed-loop TimelineSim iteration) can be
# embedded here; None falls back to the static list schedule.
_EMBED_PLAN = None


def _make_plan():
    """Static list-schedule of every engine op, mirroring the TimelineSim
    cost model (durations, completion latencies, serial HWDGE/DMA, in-order
    engines with single wait slots).  Per-engine emission order == schedule
    order, which removes head-of-line blocking on the real queues."""
    GRAN = float(_os.environ.get("KV4_GRAN", "130"))
    chunks = _feature_chunks()
    chunk_of = {}
    for bi, (need, i, fs) in enumerate(FORDER):
        col = FOFFS[(i, fs)]
        for ci, (ca, cb, cn) in enumerate(chunks):
            if ca <= col < cb:
                chunk_of[(i, fs)] = ci
                break

    ops = {}  # id -> (engine, dur, lat, deps)
    CAL = float(_os.environ.get("KV5_CAL", "1.0"))
    XLAT = float(_os.environ.get("KV5_XLAT", "0"))

    def add(oid, eng, dur, lat, deps):
        if eng in ("PE", "DVE", "ACT", "GP"):
            dur, lat = dur * CAL, lat + XLAT
        ops[oid] = (eng, dur, lat, list(deps))

    add(("ones",), "GP", 95 + 0.8333 * MAXG * B, 45, [])
    # DMA pre-pass: fixed need-order on the serial HWDGE; completions are
    # constants for the compute list-schedule below.  Pool-issued (SWDGE)
    # chunks are GP ops instead: descriptor gen on the Pool engine, then a
    # transfer slotted into the DMA-engine timeline.
    dma_seq = [("ct",), ("pi",)]
    dma_need = {("ct",): 0.0, ("pi",): float(_os.environ.get("KV7_PIN", "4150"))}
    dma_bytes = {("ct",): 128 * CT_W * 2, ("pi",): 128 * B * 2}
    for ci, (ca, cb, cn) in enumerate(chunks):
        oid = ("chunk", ci)
        dma_need[oid] = cn
        dma_bytes[oid] = 128 * (cb - ca)
        dma_seq.append(oid)
    dma_seq.sort(key=lambda o: dma_need[o])
    for oid in dma_seq:
        add(oid, "DMA", dma_bytes[oid] / 360.0, 900, [])
    def _slot_impl(i, s):
        """Multiply impl of E slot s: V / G / W (warmup = ACT copy only)."""
        if s == 0 and not _is_fw(i):
            return "W"
        return _IMPL[(i, s)]

    # shared rank-1 slot-1 operands for ones-init streams: the pre-feature
    # state is E^T(E^T 1) broadcast across columns, host-precomputed into
    # CT; one ACT broadcast each replaces every warmup matmul/copy and
    # every slot-1 matmul.
    add(("c2f",), "DVE", 130, 105, [("ct",)])
    add(("w2",), "ACT", 0.8333 * 512 + 370, 188, [("c2f",), ("ones",)])
    add(("w2x",), "ACT", 0.8333 * 512 + 370, 188, [("c2f",), ("ones",)])

    for i, (kind, g, l) in enumerate(STREAMS):
        wd = g * B
        fw = _is_fw(i)
        # last producer of the state entering slot s (s >= 1)
        if kind in "EVG":
            def st_prod(s, i=i):
                im = _slot_impl(i, s - 1)
                return [({"V": "mul", "G": "gmul"}[im], i, s - 1)]
        else:
            def st_prod(s, i=i):
                return [("exp", i, s)]
        for s in range(l + 1):
            onesinit = not fw
            if kind == "X" and s > 1:
                dep = [("ida", i, s - 1), ("idb", i, s - 1)]
                add(("exp", i, s), "ACT", 0.8333 * wd + 143, 188, dep)
            if s == W and fw:  # ones-init start colsums are host constants
                add(("sum", i, 0), "PE", 20 * g, 218, st_prod(s))
            if s == l:
                add(("sum", i, 1), "PE", 20 * g, 218, st_prod(s))
                continue
            if s == 0 and onesinit:
                continue  # warmup folded into the host-computed c2
            ft = [("chunk", chunk_of[(i, _fslot(i, s))])]
            if kind in "EVG":
                im = _slot_impl(i, s)
                first = s == 1 and onesinit  # q = c2 broadcast, no matmul
                if not first:
                    dep = [("ct",)] + (st_prod(s) if s else
                                       [("ones",), ("pi",)])
                    add(("mm", i, s), "PE", 0.4167 * wd, 218, dep)
                if im == "V":
                    qdep = [("w2",)] if first else [("mm", i, s)]
                    add(("mul", i, s), "DVE",
                        1.0417 * wd + (60 if first else 125),
                        170, qdep + ft)
                else:
                    if first:
                        add(("gmul", i, s), "GP", 95 + 1.984 * wd,
                            45, [("w2",)] + ft)
                    else:
                        add(("copy", i, s), "ACT", 0.8333 * wd + 143, 188,
                            [("mm", i, s)])
                        add(("gmul", i, s), "GP", 95 + 1.984 * wd,
                            45, [("copy", i, s)] + ft)
            else:
                first = s == 1 and onesinit  # ln(q) = lnc2x broadcast
                if not first:
                    dep = [("ct",)] + st_prod(s)
                    add(("mm", i, s), "PE", 0.4167 * wd, 218, dep)
                    add(("ln", i, s), "ACT", 0.8333 * wd + 143, 188,
                        [("mm", i, s)])
                lndep = [("w2x",)] if first else [("ln", i, s)]
                add(("ida", i, s), "PE", 0.4167 * wd, 218, lndep)
                add(("idb", i, s), "PE", 0.4167 * wd, 218,
                    [("ida", i, s)] + ft)
    # all-G bank-sharing pairs: force strict alternation (WAR via the
    # shared tile tag's single-buffer rotation)
    _gonly = [i for i in E_IDX
              if all(_IMPL[(i, s)] == "G" for s in range(STREAMS[i][2]))]
    def _qfree(i, s):
        # the op whose completion releases stream i's PSUM q of slot s
        return ("mul" if _slot_impl(i, s) == "V" else "copy", i, s)

    for j in range(1, len(_gonly), 2):
        a, b = _gonly[j - 1], _gonly[j]
        la, lb = STREAMS[a][2], STREAMS[b][2]
        for s in range(2, max(la, lb)):  # ones-init slots 0/1 have no q
            if s < lb and s < la:
                ops[("mm", b, s)][3].append(_qfree(a, s))
            if s + 1 < la and s < lb:
                ops[("mm", a, s + 1)][3].append(_qfree(b, s))
    # stage/out in two pieces: early-finishing streams' sums are staged
    # and DMA'd while the stragglers still compute; only a small final
    # piece sits on the critical tail.  The split (and the smt column
    # remap) comes from a provisional schedule pass.
    prov = bool(_os.environ.get("KV7_PROV"))
    if not prov:
        ns = len(STREAMS)
        sum_done = _prov_sum_done()
        forder_s = sorted(range(ns), key=lambda i: sum_done[i])
        smb, acc = {}, 0
        for i in forder_s:
            smb[i] = acc
            acc += STREAMS[i][1]
        late = [i for i in range(ns)
                if sum_done[i] > max(sum_done.values()) - float(_os.environ.get("KV7_LATE", "600"))]
        early = [i for i in range(ns) if i not in late]
        cut = 8 * min(smb[i] for i in late)  # late streams = column suffix
        esums = [("sum", i, ev) for i in early for ev in (0, 1)]
        esums = [o for o in esums if o in ops]
        lsums = [("sum", i, ev) for i in late for ev in (0, 1)]
        lsums = [o for o in lsums if o in ops]
        _stg = _os.environ.get("KV7_STG", "dve,dve").split(",")
        _se = {"act": "ACT", "dve": "DVE"}
        add(("stage", 0), _se[_stg[0]], 0.8333 * cut + 143, 188, esums)
        add(("outdma", 0), "DMA", 128 * cut * 4 / 360.0, 900,
            [("stage", 0)])
        add(("stage", 1), _se[_stg[1]], 0.8333 * (8 * CPC - cut) + 143,
            188, lsums)
        add(("outdma", 1), "DMA", 128 * (8 * CPC - cut) * 4 / 360.0, 900,
            [("stage", 1)])
        globals()["_SMB"] = smb
        globals()["_SMT_CUT"] = cut

    # priorities: longest path to the sink
    prio = {}

    children = {k: [] for k in ops}
    for oid, (_, _, _, deps) in ops.items():
        for d in deps:
            children[d].append(oid)

    def get_prio(oid):
        if oid in prio:
            return prio[oid]
        eng, dur, lat, _ = ops[oid]
        p = dur + max((get_prio(c) for c in children[oid]), default=lat)
        prio[oid] = p
        return p

    import sys as _sys
    _sys.setrecursionlimit(10000)
    for oid in ops:
        get_prio(oid)

    # two-pass greedy list schedule (second pass re-rates matmuls that land
    # before the PE clock finishes ramping at ~3.1us)
    mid_mm = set()
    for _pass in range(2):
        done = {}
        free = {"PE": 650.0, "DVE": 650.0, "ACT": 650.0, "GP": 650.0}
        start_t = {}
        # DMA pre-pass (fixed order, serial HWDGE then serial transfer).
        # DMA-engine busy intervals kept for gap-fitting pool-DMA transfers.
        hw = 691.0
        dmae_busy = []
        dmae = 691.0
        for oid in dma_seq:
            start_t[oid] = hw
            hw += 625.0
            tr = max(hw + 650.0, dmae)
            dmae = tr + ops[oid][1]
            dmae_busy.append((tr, dmae))
            done[oid] = dmae + 900.0

        def dmae_fit(ready, dur):
            # first-fit into gaps of the DMA-engine timeline
            t = ready
            for s, e in sorted(dmae_busy):
                if t + dur <= s:
                    break
                if t < e:
                    t = e
            dmae_busy.append((t, t + dur))
            return t + dur

        todo = sorted(set(ops) - set(dma_seq), key=repr)  # deterministic
        while todo:
            best = None
            for oid in todo:
                eng, dur, lat, deps = ops[oid]
                if any(d not in done for d in deps):
                    continue
                rt = max([done[d] for d in deps], default=650.0)
                rt = max(rt, 650.0)
                st = max(rt, hw if eng == "DMA" else free[eng])
                key = (int(st / GRAN), -prio[oid], st)
                if best is None or key < best[0]:
                    best = (key, oid, st)
            _, oid, st = best
            eng, dur, lat, deps = ops[oid]
            if oid in mid_mm:
                dur = dur * 2
            if eng == "DMA":  # outdma
                hw = st + 625.0
                tr = max(hw + 650.0, dmae)
                dmae = tr + dur
                done[oid] = dmae + 900.0
            elif oid[0] == "chunk":  # pool-issued SWDGE dma: GP desc-gen
                end = st + dur
                free["GP"] = end
                tr_end = dmae_fit(end + 650.0,
                                  dma_bytes[oid] / 360.0)
                done[oid] = tr_end + 900.0
            elif eng == "PE":
                end = st + dur
                free[eng] = end
                done[oid] = max(st + 218, end + 45)
            else:
                end = st + dur
                free[eng] = end
                done[oid] = end + lat
            start_t[oid] = st
            todo.remove(oid)
        mid_mm = {oid for oid in ops
                  if ops[oid][0] == "PE" and start_t[oid] < 3100
                  and oid[0] in ("mm", "ida", "idb")}
    if prov:
        _PROV_CACHE["_PROV"] = {
            i: done[("sum", i, 1)] for i in range(len(STREAMS))}
    if _os.environ.get("KV4_DBG"):
        mk = max(done.values())
        print(f"[plan] predicted makespan: {mk:.0f} ns")
    globals()["_PLAN_STARTS"] = dict(start_t)  # for calibration tooling
    globals()["_PLAN_DONE"] = dict(done)
    plan = sorted(ops, key=lambda oid: (start_t[oid], -prio[oid]))
    def _reorder(order):
        # sort by the given rank, then topologically repair (ties and
        # trace-mapping jitter can locally invert producer/consumer)
        rank = {o: r for r, o in enumerate(order)}
        cand = sorted(plan, key=lambda oid: rank.get(_norm_oid(oid), 10**9))
        emitted, out, pend = set(), [], list(cand)
        while pend:
            moved = False
            rest = []
            for oid in pend:
                if all(d in emitted for d in ops[oid][3]):
                    emitted.add(oid)
                    out.append(oid)
                    moved = True
                else:
                    rest.append(oid)
            pend = rest
            assert moved, "plan order has a dependency cycle"
        return out

    pf = _os.environ.get("KV6_PLANFILE")
    if pf:
        import json as _json
        with open(pf) as f:
            order = [tuple(o) for o in _json.load(f)]
        assert set(order) == set(map(_norm_oid, plan)), "plan op set mismatch"
        plan = _reorder(order)
    elif _EMBED_PLAN is not None:
        order = [tuple(o) for o in _EMBED_PLAN]
        if set(order) == set(map(_norm_oid, plan)):
            plan = _reorder(order)
    return plan, chunks


@with_exitstack
def _body(ctx, tc, OUT_d, CT_d, PI_d, F_d):
    nc = tc.nc
    const = ctx.enter_context(tc.tile_pool(name="const", bufs=1))
    fpool = ctx.enter_context(tc.tile_pool(name="f", bufs=1))
    spool = ctx.enter_context(tc.tile_pool(name="s", bufs=int(_os.environ.get("KV2_SB", "12"))))
    qspool = ctx.enter_context(tc.tile_pool(name="qs", bufs=int(_os.environ.get("KV2_QB", "8"))))
    xpool = ctx.enter_context(tc.tile_pool(name="x", bufs=int(_os.environ.get("KV2_XB", "4"))))
    # q pools: streams with any V slot keep q live until the DVE mul reads
    # it (most of the slot cadence) -> own bank.  All-G streams' q is live
    # only matmul->ACT-copy, so pairs of them share one bank.
    dq = {}
    _gonly = [i for i in E_IDX
              if all(_IMPL[(i, s)] == "G" for s in range(STREAMS[i][2]))]
    _vany = [i for i in E_IDX if i not in _gonly]
    for i in _vany:
        dq[i] = ctx.enter_context(
            tc.tile_pool(name=f"dq{i}", bufs=1, space=bass.MemorySpace.PSUM))
    qtag = {i: f"q{i}" for i in _vany}
    for j, i in enumerate(_gonly):
        if j % 2 == 1:
            dq[i] = dq[_gonly[j - 1]]
            qtag[i] = f"q{_gonly[j - 1]}"
        else:
            dq[i] = ctx.enter_context(
                tc.tile_pool(name=f"dq{i}", bufs=1,
                             space=bass.MemorySpace.PSUM))
            qtag[i] = f"q{i}"
    xq = ctx.enter_context(
        tc.tile_pool(name="xq", bufs=1, space=bass.MemorySpace.PSUM))
    smpool = ctx.enter_context(
        tc.tile_pool(name="sm", bufs=1, space=bass.MemorySpace.PSUM))

    # One act table serves Copy+Ln+Exp; without this the table-load pass
    # thrashes 1.3us loads between per-func default tables.
    nc.scalar.add_instruction(
        mybir.InstLoadActFuncSet(
            name=nc.get_next_instruction_name(), ins=[], outs=[],
            act_func_set_id=6,  # natural_log_exp_and_others
        )
    )
    bxt = const.tile([T2, 1], F32, tag="bx")
    nc.vector.memset(bxt[:], BX)

    ct = const.tile([T2, CT_W], BF16, tag="consts")
    nc.sync.dma_start(ct[:], CT_d[:])
    et = ct[:, CT_ET : CT_ET + 128]
    ge = ct[:, CT_GE : CT_GE + 2]
    idm = ct[:, CT_ID : CT_ID + 128]
    ones = const.tile([T2, MAXG * B], BF16, tag="ones")
    nc.gpsimd.memset(ones[:], 1.0)
    pinit = const.tile([T2, B], BF16, tag="pinit")
    # No PE preheat: the cost model's p-state clock ramps from the engine's
    # FIRST activity (the entry drain at t~70ns), so the PE hits full speed
    # at t~3.1us -- exactly when the first DMA'd operands can arrive
    # (HWDGE 625 + dge 650 + transfer + 900 sem after the ~650ns entry
    # barrier).  Dummy warmup matmuls only delay real work.
    NPH = int(_os.environ.get("KV3_PH", "0"))
    PHW = int(_os.environ.get("KV3_PHW", "512"))
    if NPH:
        hg = STREAMS[D_IDX[-1]][1]
        qheat = dq[D_IDX[-1]].tile([T2, hg * B], F32, tag=f"q{D_IDX[-1]}")
        for _ in range(NPH):
            nc.tensor.matmul(qheat[:, 0:PHW], ones[:, 0:128], ones[:, 0:PHW],
                             start=True, stop=True)

    # Features: all SBUF-resident, packed into ONE need-ordered tile so a
    # single DMA range feeds many streams' upcoming slots.
    fall = fpool.tile([T2, FCOLS], FP8, tag="fall")

    def ftv(i, s):
        off = FOFFS[(i, _fslot(i, s))]
        return fall[:, off : off + STREAMS[i][1] * B]

    # per-slot tile records, filled as the plan emits
    state = {}   # (i, s) -> state tile entering slot s
    qtile = {}   # (i, s) -> PSUM q
    qstile = {}  # (i, s) -> bf16 SBUF copy of q (P)
    lntile = {}  # (i, s) -> lnq (X)
    xatile = {}  # (i, s) -> xa PSUM (X)
    for i, (kind, g, l) in enumerate(STREAMS):
        state[(i, 0)] = ones[:, 0 : g * B]

    # transposed sums: per (chain, delta/end) event 4 columns of one PSUM
    # tile: [colsum_h0, wy_h0, colsum_h1, wy_h1], batch = partition row.
    smt = smpool.tile([T2, 8 * CPC], F32, tag="smt")
    stage = [None]

    def em_mm(i, s):
        kind, g, l = STREAMS[i]
        wd = g * B
        src = state[(i, s)]
        if kind in "EVG":
            q = dq[i].tile([T2, wd], F32, tag=qtag[i], name=f"q_{i}_{s}")
            if _is_fw(i) and s == 0:  # block0 exact init, block1 ones
                nc.tensor.matmul(q[:, 0:B], et[:], pinit[:],
                                 start=True, stop=True)
                nc.tensor.matmul(q[:, B:wd], et[:], src[:, B:wd],
                                 start=True, stop=True)
            else:
                nc.tensor.matmul(q[:], et[:], src[:], start=True, stop=True)
        else:
            q = xq.tile([T2, wd], F32, tag=f"xq{i}", name=f"q_{i}_{s}")
            for h in range(0, wd, 512):  # per-PSUM-bank matmuls
                nc.tensor.matmul(q[:, h : h + 512], et[:],
                                 src[:, h : h + 512], start=True, stop=True)
        qtile[(i, s)] = q

    w2 = [None, None]  # [w2, w2x] broadcast tiles
    c2f = const.tile([T2, 2], F32, tag="c2f")  # activation scale must be f32

    def em_c2f():
        nc.vector.tensor_copy(c2f[:], ct[:, CT_C2 : CT_C2 + 2])

    def em_w2(which):
        t = const.tile([T2, MAXG * B], BF16, tag=f"w2_{which}",
                       name=f"w2_{which}")
        nc.scalar.activation(t[:], ones[:, 0 : MAXG * B], AF.Copy,
                             scale=c2f[:, which : which + 1])
        w2[which] = t

    def em_mul(i, s):
        wd = STREAMS[i][1] * B
        sn = spool.tile([T2, wd], BF16, tag=f"s{i}", name=f"sn_{i}_{s}")
        qsrc = (w2[0][:, 0:wd] if (s == 1 and not _is_fw(i))
                else qtile[(i, s)][:])
        nc.vector.tensor_mul(sn[:], qsrc, ftv(i, s))
        state[(i, s + 1)] = sn

    def em_copy(i, s):
        wd = STREAMS[i][1] * B
        qs = qspool.tile([T2, wd], BF16, tag=f"qs{i}", name=f"qs_{i}_{s}")
        nc.scalar.copy(qs[:], qtile[(i, s)][:])
        qstile[(i, s)] = qs
        if s == 0 and not _is_fw(i):  # warmup: the copy IS the next state
            state[(i, 1)] = qs

    def em_gmul(i, s):
        wd = STREAMS[i][1] * B
        sn = spool.tile([T2, wd], BF16, tag=f"s{i}", name=f"sn_{i}_{s}")
        qsrc = (w2[0][:, 0:wd] if (s == 1 and not _is_fw(i))
                else qstile[(i, s)][:])
        nc.gpsimd.tensor_mul(sn[:], qsrc, ftv(i, s))
        state[(i, s + 1)] = sn

    def em_ln(i, s):
        wd = STREAMS[i][1] * B
        lnq = xpool.tile([T2, wd], BF16, tag=f"lnq{i}", name=f"lnq_{i}_{s}")
        nc.scalar.activation(lnq[:], qtile[(i, s)][:], AF.Ln)
        lntile[(i, s)] = lnq

    def em_ida(i, s):
        wd = STREAMS[i][1] * B
        lnsrc = (w2[1] if (s == 1 and not _is_fw(i))
                 else lntile[(i, s)])
        xa = xq.tile([T2, wd], F32, tag=f"xq{i}", name=f"xa_{i}_{s}")
        for h in range(0, wd, 512):
            nc.tensor.matmul(xa[:, h : h + 512], idm[:],
                             lnsrc[:, h : h + 512],
                             start=True, stop=False)
        xatile[(i, s)] = xa

    def em_idb(i, s):
        wd = STREAMS[i][1] * B
        xa = xatile[(i, s)]
        for h in range(0, wd, 512):
            nc.tensor.matmul(xa[:, h : h + 512], idm[:],
                             ftv(i, s)[:, h : h + 512],
                             start=False, stop=True)

    def em_exp(i, s):
        wd = STREAMS[i][1] * B
        e = xpool.tile([T2, wd], BF16, tag=f"e{i}", name=f"e_{i}_{s}")
        nc.scalar.activation(e[:], xatile[(i, s - 1)][:], AF.Exp,
                             bias=bxt[:], scale=1.0)
        state[(i, s)] = e

    def em_sum(i, ev):
        kind, g, l = STREAMS[i]
        stt = state[(i, W if ev == 0 else l)]
        for gg in range(g):
            e4 = (2 * (_SMB[i] + gg) + ev) * 4
            for h in range(2):
                nc.tensor.matmul(
                    smt[:, e4 + 2 * h : e4 + 2 * h + 2],
                    stt[:, gg * B + h * T2 : gg * B + (h + 1) * T2],
                    ge[:], start=True, stop=True)

    def em_stage(k):
        if stage[0] is None:
            stage[0] = const.tile([T2, 8 * CPC], F32, tag="stage",
                                  name="stage")
        a, b = (0, _SMT_CUT) if k == 0 else (_SMT_CUT, 8 * CPC)
        eng = _os.environ.get("KV7_STG", "dve,dve").split(",")[k]
        if eng == "act":
            nc.scalar.copy(stage[0][:, a:b], smt[:, a:b])
        else:
            nc.vector.tensor_copy(stage[0][:, a:b], smt[:, a:b])

    plan, chunks = _make_plan()
    _SMB, _SMT_CUT = globals()["_SMB"], globals()["_SMT_CUT"]
    for oid in plan:
        k = oid[0]
        if k == "ones":
            pass  # emitted above
        elif k == "ct":
            pass  # emitted above (needed before et/ge/idm slices)
        elif k == "pi":
            nc.sync.dma_start(pinit[:], PI_d[:])
        elif k == "chunk":
            ca, cb, cn = chunks[oid[1]]
            nc.sync.dma_start(fall[:, ca:cb], F_d[:, ca:cb])
        elif k == "c2f":
            em_c2f()
        elif k == "w2":
            em_w2(0)
        elif k == "w2x":
            em_w2(1)
        elif k == "mm":
            em_mm(oid[1], oid[2])
        elif k == "mul":
            em_mul(oid[1], oid[2])
        elif k == "copy":
            em_copy(oid[1], oid[2])
        elif k == "gmul":
            em_gmul(oid[1], oid[2])
        elif k == "ln":
            em_ln(oid[1], oid[2])
        elif k == "ida":
            em_ida(oid[1], oid[2])
        elif k == "idb":
            em_idb(oid[1], oid[2])
        elif k == "exp":
            em_exp(oid[1], oid[2])
        elif k == "sum":
            em_sum(oid[1], oid[2])
        elif k == "stage":
            em_stage(oid[1])
        elif k == "outdma":
            a, b = (0, _SMT_CUT) if oid[1] == 0 else (_SMT_CUT, 8 * CPC)
            nc.sync.dma_start(OUT_d[:, a:b], stage[0][:, a:b])


_NC_CACHE = {}


def _get_nc():
    if "nc" not in _NC_CACHE:
        nc = bacc.Bacc("TRN2", target_bir_lowering=False, debug=False)
        CT_d = nc.dram_tensor("CT", [T2, CT_W], BF16, kind="ExternalInput")
        PI_d = nc.dram_tensor("PI", [T2, B], BF16, kind="ExternalInput")
        F_d = nc.dram_tensor("FALL", [T2, FCOLS], FP8,
                             kind="ExternalInput")
        OUT_d = nc.dram_tensor("OUT", [T2, 8 * CPC], F32, kind="ExternalOutput")
        with tile.TileContext(nc) as tc:
            _body(tc, OUT_d, CT_d, PI_d, F_d)
        nc.compile()
        _NC_CACHE["nc"] = nc
    return _NC_CACHE["nc"]


def _host_fold(feats, trans):
    """Exact f64 log-domain forward for steps 0..KHOST-1.
    Returns (init_bf16 [T2,B] = exp(alpha-m), m [B])."""
    E = np.exp(trans.astype(np.float64))
    alpha = np.full((B, T2), -1e5, np.float64)
    alpha[:, -1] = 0.0
    for t in range(KHOST):
        mm = alpha.max(-1, keepdims=True)
        alpha = mm + np.log(np.exp(alpha - mm) @ E.T) + feats[:, t, :]
    m = alpha.max(-1)
    init = np.exp(alpha - m[:, None]).T  # [T2, B]
    return init.astype(NPBF), m


def prepare_in_maps(feats, trans):
    feats = np.asarray(feats, dtype=np.float32)
    trans = np.asarray(trans, dtype=np.float32)
    assert feats.shape == (B, S, T2) and trans.shape == (T2, T2)

    with np.errstate(under="ignore", over="ignore"):
        # floor: blocked transitions give q=0 -> Ln=-inf -> 0*inf=NaN in the
        # identity matmul on the log path; 1e-30 keeps everything finite and
        # contributes ~e^-67 to colsums (negligible)
        ET = np.maximum(np.exp(trans).T, 1e-30)   # [from, to]
        F8 = np.exp(feats.transpose(2, 1, 0) - SHIFT).astype(NPF8)  # [T2,S,B]
    FXf = feats.transpose(2, 1, 0).astype(NPF8)   # raw (log domain), fp8
    init, m0 = _host_fold(feats, trans)

    CT = np.zeros((T2, CT_W), np.float32)
    CT[:, CT_ET : CT_ET + 128] = ET
    CT[:, CT_GE] = 1.0
    CT[:, CT_GE + 1] = np.exp(trans[-2, :])
    CT[:, CT_ID : CT_ID + 128] = np.eye(T2)
    c1 = ET.sum(axis=0)
    c2 = ET.T @ c1
    CT[:, CT_C2] = c2
    CT[:, CT_LX] = np.log(c2) + BX
    CT = CT.astype(NPBF)
    # host-side start colsums for ones-init chains (state1 = c1 broadcast)
    _NC_CACHE["lnS0_E"] = float(np.log(np.float64(c1).sum()))
    _NC_CACHE["lnS0_X"] = float(np.log(np.float64(c1).sum()) + BX)
    PI = np.ones((NCORES, T2, B), NPBF)
    PI[0] = init

    in_maps = []
    ci = 0
    for k in range(NCORES):
        mp = {"CT": CT, "PI": PI[k]}
        fallbuf = np.zeros((T2, FCOLS), NPF8)
        for i, (kind, g, l) in enumerate(STREAMS):
            fl = _flen(i)
            t0s = [STARTS[ci + gg] - (W if _is_fw(i) else 0)
                   for gg in range(g)]
            srcf = FXf if kind == "X" else F8
            for fs in range(fl):
                off = FOFFS[(i, fs)]
                for gg, t0 in enumerate(t0s):
                    fallbuf[:, off + gg * B : off + (gg + 1) * B] = (
                        srcf[:, t0 + fs, :])
            ci += g
        mp["FALL"] = fallbuf
        in_maps.append(mp)
    _NC_CACHE["m0"] = m0
    return in_maps


def postprocess(results):
    m0 = _NC_CACHE["m0"]
    logZ = m0.astype(np.float64).copy()
    ci = 0
    with np.errstate(divide="ignore"):
        for k, r in enumerate(results):
            out = r["OUT"].astype(np.float64)

            def col(e, j):  # j: 0=colsum 1=w.y -> [B]
                return np.concatenate([out[:, 4 * e + j], out[:, 4 * e + 2 + j]])

            smb = globals()["_SMB"]
            for si, (kind, g, l) in enumerate(STREAMS):
                for gg in range(g):
                    bi = smb[si] + gg
                    last = ci == NCH - 1
                    logZ += np.log(col(2 * bi + 1, 1 if last else 0))
                    if ci > 0:
                        if _is_fw(si):
                            logZ -= np.log(col(2 * bi, 0))
                        else:  # ones-init: start colsum is a host constant
                            logZ -= (_NC_CACHE["lnS0_X"] if kind == "X"
                                     else _NC_CACHE["lnS0_E"])
                        nsl = l - W
                    else:
                        nsl = l  # chain 0: warmup slots are real
                    logZ += (-BX if kind == "X" else SHIFT) * nsl
                    ci += 1
    return logZ.astype(np.float32)


def run(feats, trans, trace=False, **spmd_kwargs):
    nc = _get_nc()
    in_maps = prepare_in_maps(feats, trans)
    res = run_bass_kernel_spmd(
        nc, in_maps, list(range(NCORES)), trace=trace, **spmd_kwargs
    )
    return postprocess(res.results), res


def kernel(feats, trans):
    out, _ = run(feats, trans, trace=False)
    return out
